# revision 22
# baseline (speedup 1.0000x reference)
"""Trainium2 Bass kernel for nn_CUDAOptimizedBKCore: diagonal Green's function
of a complex-shifted tridiagonal matrix via forward/backward continuant
recursions, data-parallel over the batch across 8 NeuronCores.

v1 design:
- Segmented pow2 rescaling: recursion runs unrescaled in fp16 (3 plain
  tensor_tensor ops/step, all 2x DVE mode); carries are scaled by an exact
  2^-11 every 16 steps. With sigma(j)=j//16 the combine scale
  sigma(79-m)+sigma(m)=4 is constant, folded into the phi seed.
  Validated offline: max rel err 1.15e-2.
- Host precomputes A2 = (-(h0+he), +(h0+he)) in fp16 in the exact SBUF tile
  layout [P, n, 2, f]; host upcasts the fp16 g output.
- Combine G = tau (x) pi in fp16, batched CB steps at a time; GPSIMD takes
  batches during the phi pass, DVE takes the rest; g stored fp16 in DRAM.
"""
import numpy as np

_CACHE = {}

import concourse.bass as bass
import concourse.bacc as bacc
import concourse.tile as tile
from concourse import mybir

F32 = mybir.dt.float32
F16 = mybir.dt.float16
P = 128
SC = float(2.0 ** -11)   # exact pow2 segment rescale (every 16 steps)
CB = 8            # combine batch (phi steps per batched combine)
GCHUNK = 16       # g k-columns per DMA chunk


def build_nc(b_core: int, n: int, f: int, n_cores: int = 8, loops: int = 1,
             gps_batches=(0, 1, 2, 3, 4, 6), f_g: int = 0, halves: bool = False,
             u_gps_th=(), u_gps_ph=()):
    """Bacc program for one core's slice.

    Inputs: a2 [b=P*f rows as [P, n, 2, f] fp16] = -+a/s; output g_dev
    [P, n//GCHUNK, f, GCHUNK, 2] fp16 (host reorders + upcasts).
    gps_batches: combine batches assigned to GPSIMD (rest on DVE).
    f_g: rows per partition owned end-to-end by GPSIMD side-chains; the
    remaining f-f_g rows run as two interleaved DVE-only half-chains.
    No cross-engine dependencies inside the scan loops.
    """
    assert b_core == P * f
    assert n % CB == 0 and GCHUNK % CB == 0
    nc = bacc.Bacc("TRN2", target_bir_lowering=False, debug=False, num_devices=n_cores)
    a2_d = nc.dram_tensor("a2", [P, n * 2 * f], F16, kind="ExternalInput").ap()
    g_d = nc.dram_tensor("g", [P, (n // GCHUNK) * f * GCHUNK * 2], F16,
                         kind="ExternalOutput").ap()

    mult, add = mybir.AluOpType.mult, mybir.AluOpType.add

    with tile.TileContext(nc) as tc:
        with (
            tc.tile_pool(name="coef", bufs=1) as coef,
            tc.tile_pool(name="hist", bufs=1) as hist,
            tc.tile_pool(name="gring", bufs=3) as gring,
            tc.tile_pool(name="tmp", bufs=4) as tmp,
            tc.tile_pool(name="qpool", bufs=2) as qpool,
        ):
            KCH = 16
            a2_5d = a2_d.rearrange("p (q k c f) -> p q k c f", q=n // KCH, k=KCH, c=2)

            import contextlib
            loop_cm = tc.For_i(0, loops, 1) if loops > 1 else contextlib.nullcontext()
            with loop_cm:
                # A2 loaded in k-chunks (inside the loop: each iteration is a
                # full kernel execution incl. input DMA); scan starts after
                # chunk 0 lands.
                A2c = []
                for q in range(n // KCH):
                    t_ = coef.tile([P, KCH, 2, f], F16, name=f"a2c{q}")
                    nc.sync.dma_start(out=t_[:], in_=a2_5d[:, q])
                    A2c.append(t_)
                # phi reads chunk 0 last (reversed coefficient order); give it
                # a private copy so the main chunk-0 tile frees early in theta
                # and the next loop iteration's input DMA can prefetch.
                A2p0 = coef.tile([P, KCH, 2, f], F16, name="a2p0")
                nc.sync.dma_start(out=A2p0[:], in_=a2_5d[:, 0])

                def A2(t):
                    return A2c[t // KCH][:, t % KCH]

                def A2phi(t):
                    return A2p0[:, t] if t < KCH else A2(t)

                TH = hist.tile([P, n + 1, 2, f], F16)   # tau history (theta pass)
                PH = hist.tile([P, n, 2, f], F16)       # pi history (phi pass)
                Cth = hist.tile([P, 2, f], F16)         # rescaled carry (theta)
                Cph = hist.tile([P, 2, f], F16)         # rescaled carry (phi)

                # ---- theta init: tau_0=(1,0), tau_1=(1/s, a0/s), y_1 = tau_1 + tau_0/s
                nc.vector.memset(TH[:, 0, 0], 1.0)
                nc.vector.memset(TH[:, 0, 1], 0.0)
                nc.vector.memset(TH[:, 1, 0], 1.0)
                nc.vector.tensor_copy(out=TH[:, 1, 1], in_=A2(0)[:, 1])

                f_d = f - f_g                   # DVE-owned rows
                fh = f_d // 2
                # row slices: DVE chains (1 or 2) + optional GPSIMD side-chain
                chains = []
                if f_d > 0:
                    if halves:
                        chains.append((nc.vector, slice(0, fh)))
                        chains.append((nc.vector, slice(fh, f_d)))
                    else:
                        chains.append((nc.vector, slice(0, f_d)))
                if f_g > 0:
                    chains.append((nc.gpsimd, slice(f_d, f)))

                def step(src, dst, prev, a_k, boundary, C, carry_next, a2fn=A2,
                         u_gps=()):
                    """dst = src + prev + A2[a_k] o src_sw  (3 plain TT, 2x mode).

                    boundary: scale dst (stored history) by exact 2^-11 in
                    place and emit the rescaled carry C = src * 2^-11 for the
                    next step. u_gps: chain indices whose u-add runs on GPSIMD
                    (u has half a step of slack before n consumes it). All m
                    ops are emitted before the n ops so a waiting n does not
                    head-of-line-block ready work on the in-order DVE queue."""
                    ms, us = [], []
                    for idx, (eng, sl) in enumerate(chains):
                        w = sl.stop - sl.start
                        m = tmp.tile([P, 2, w], F16, tag=f"m{sl.start}", name="m")
                        eng.tensor_tensor(
                            out=m[:], in0=a2fn(a_k)[:, :, sl], in1=src[:, ::-1, sl],
                            op=mult)
                        ms.append(m)
                    for idx, (eng, sl) in enumerate(chains):
                        w = sl.stop - sl.start
                        ueng = nc.gpsimd if idx in u_gps else eng
                        u = tmp.tile([P, 2, w], F16, tag=f"u{sl.start}", name="u")
                        ueng.tensor_add(out=u[:], in0=src[:, :, sl], in1=prev[:, :, sl])
                        us.append(u)
                    for idx, (eng, sl) in enumerate(chains):
                        eng.tensor_add(out=dst[:, :, sl], in0=us[idx][:], in1=ms[idx][:])
                        if boundary:
                            eng.tensor_scalar_mul(
                                out=dst[:, :, sl], in0=dst[:, :, sl], scalar1=SC)
                            if carry_next:
                                eng.tensor_scalar_mul(
                                    out=C[:, :, sl], in0=src[:, :, sl], scalar1=SC)

                # ---- theta pass ----
                prev_is_c = False
                for t in range(1, n):
                    j = t + 1
                    boundary = (j % 16 == 0)
                    prev = Cth if prev_is_c else TH[:, t - 1]
                    step(TH[:, t], TH[:, t + 1], prev, t, boundary, Cth, j < n,
                         u_gps=u_gps_th)
                    prev_is_c = boundary and j < n

                # ---- seed pi_0 = (i/s)/tau_N (fp32 math on DVE), per row-slice;
                # GPSIMD's slice seeded first so its phi chain starts ASAP.
                def seed(sl):
                    w = sl.stop - sl.start
                    TN32 = tmp.tile([P, 2, w], F32, tag=f"w0{sl.start}", name="TN32")
                    nc.vector.tensor_copy(out=TN32[:], in_=TH[:, n, :, sl])
                    dr32 = tmp.tile([P, w], F32, tag=f"w{sl.start}", name="dr32")
                    nc.vector.tensor_tensor(out=dr32[:], in0=TN32[:, 0], in1=TN32[:, 0], op=mult)
                    di32 = tmp.tile([P, w], F32, tag=f"w2{sl.start}", name="di32")
                    nc.vector.tensor_tensor(out=di32[:], in0=TN32[:, 1], in1=TN32[:, 1], op=mult)
                    nc.vector.tensor_add(out=dr32[:], in0=dr32[:], in1=di32[:])
                    inv = tmp.tile([P, w], F32, tag=f"w3{sl.start}", name="inv")
                    nc.vector.reciprocal(out=inv[:], in_=dr32[:])
                    nc.vector.tensor_scalar_mul(out=inv[:], in0=inv[:], scalar1=SC)
                    # pi_0 = i*2^-11 / T~_N   (fp16 out)
                    nc.vector.tensor_tensor(out=PH[:, 0, 0, sl], in0=TN32[:, 1], in1=inv[:], op=mult)
                    nc.vector.tensor_tensor(out=PH[:, 0, 1, sl], in0=TN32[:, 0], in1=inv[:], op=mult)
                    # pi_1 = pi_0 + A2[n-1] o pi_0_sw
                    m0 = tmp.tile([P, 2, w], F16, tag=f"m{sl.start}", name="m0")
                    nc.vector.tensor_tensor(
                        out=m0[:], in0=A2(n - 1)[:, :, sl], in1=PH[:, 0, ::-1, sl], op=mult)
                    nc.vector.tensor_add(
                        out=PH[:, 1, :, sl], in0=PH[:, 0, :, sl], in1=m0[:])

                for _, sl in reversed(chains):   # GPS slice first
                    seed(sl)

                # g chunk tiles: [P, f, GCHUNK, 2] fp16, DMA'd out when complete
                g4 = g_d.rearrange("p (q f k c) -> p q f k c", q=n // GCHUNK, f=f, k=GCHUNK)
                chunk_t = {}

                def combine_batch(b, eng):
                    """G_{n-1-m} = tau_{n-1-m} (x) pi_m for m in [CB*b, CB*b+CB)."""
                    m0i = CB * b
                    hi = (n - 1) - m0i
                    ci = (hi - CB + 1) // GCHUNK
                    if ci not in chunk_t:
                        chunk_t[ci] = gring.tile(
                            [P, f, GCHUNK, 2], F16, tag="g", name=f"gchunk{ci}")
                    gc = chunk_t[ci]
                    xs = None if hi - CB < 0 else hi - CB
                    X = TH[:, hi:xs:-1]             # (P, CB, 2, f) tau rows desc
                    Yp = PH[:, m0i:m0i + CB]        # (P, CB, 2, f)
                    Yps = PH[:, m0i:m0i + CB, ::-1]
                    tg = "g" if eng is nc.gpsimd else "v"   # per-engine bufs
                    q1 = qpool.tile([P, CB, 2, f], F16, tag=f"q1{tg}", name="q1")
                    eng.tensor_tensor(out=q1[:], in0=X, in1=Yp, op=mult)
                    q2 = qpool.tile([P, CB, 2, f], F16, tag=f"q2{tg}", name="q2")
                    eng.tensor_tensor(out=q2[:], in0=X, in1=Yps, op=mult)
                    jhi = hi - GCHUNK * ci
                    js = None if jhi - CB < 0 else jhi - CB
                    og_r = gc[:, :, jhi:js:-1, 0].transpose([0, 2, 1])
                    og_i = gc[:, :, jhi:js:-1, 1].transpose([0, 2, 1])
                    eng.tensor_sub(out=og_r[:], in0=q1[:, :, 0], in1=q1[:, :, 1])
                    eng.tensor_add(out=og_i[:], in0=q2[:, :, 0], in1=q2[:, :, 1])
                    if jhi - CB + 1 == 0:           # chunk complete -> DMA out
                        nc.sync.dma_start(out=g4[:, ci], in_=gc[:])
                        del chunk_t[ci]

                # ---- phi pass (t = 1..n-2), combines interleaved ----
                prev_is_c = False
                for t in range(1, n - 1):
                    j = t + 1
                    boundary = (j % 16 == 0)
                    prev = Cph if prev_is_c else PH[:, t - 1]
                    step(PH[:, t], PH[:, t + 1], prev, n - 1 - t, boundary,
                         Cph, True, a2fn=A2phi, u_gps=u_gps_ph)
                    prev_is_c = boundary
                    if (t + 2) % CB == 0:
                        b = (t + 2) // CB - 1
                        eng = nc.gpsimd if b in gps_batches else nc.vector
                        combine_batch(b, eng)

    nc.compile()
    return nc


def make_a2(he_diag: np.ndarray, h0_diag: np.ndarray, n_cores: int, f: int):
    """Host-side A2 = (-(h0+he), +(h0+he)) fp16, laid out [core][P, n, 2, f]."""
    B, n = he_diag.shape
    a = h0_diag[None, :].astype(np.float32) + he_diag.astype(np.float32)
    a16 = a.astype(np.float16)                    # (B, n) = +a
    b_core = B // n_cores
    out = []
    for c in range(n_cores):
        ac = a16[c * b_core:(c + 1) * b_core]     # (b_core, n)
        ac = ac.reshape(P, f, n)                  # p, f, n
        a2 = np.empty((P, n, 2, f), np.float16)
        ap = ac.transpose(0, 2, 1)                # p, n, f
        a2[:, :, 1, :] = ap
        a2[:, :, 0, :] = -ap
        out.append(np.ascontiguousarray(a2.reshape(P, n * 2 * f)))
    return out


def unpack_g(g_dev: np.ndarray, f: int, n: int) -> np.ndarray:
    """g_dev [P, (n//GCHUNK)*f*GCHUNK*2] fp16 -> (b_core, n, 2) fp32."""
    q = n // GCHUNK
    g = g_dev.reshape(P, q, f, GCHUNK, 2).astype(np.float32)
    g = g.transpose(0, 2, 1, 3, 4).reshape(P * f, n, 2)
    return g


def _get_nc(b_core, n, f, n_cores, loops=1):
    key = (b_core, n, f, n_cores, loops)
    if key not in _CACHE:
        _CACHE[key] = build_nc(b_core, n, f, n_cores=n_cores, loops=loops)
    return _CACHE[key]


def kernel(he_diag, h0_diag, h0_sub, h0_super):
    from concourse.bass_utils import run_bass_kernel_spmd

    he_diag = np.asarray(he_diag, dtype=np.float32)
    B, n = he_diag.shape
    n_cores = 8
    assert B % n_cores == 0
    b_core = B // n_cores
    assert b_core % P == 0
    f = b_core // P

    s = np.asarray(h0_super, dtype=np.float32) * np.asarray(h0_sub, dtype=np.float32)
    assert bool(np.all(s == np.float32(1.0))), "kernel assumes unit off-diagonal product"

    a2_maps = make_a2(he_diag, np.asarray(h0_diag, np.float32), n_cores, f)
    nc = _get_nc(b_core, n, f, n_cores)
    in_maps = [{"a2": a2_maps[c]} for c in range(n_cores)]
    res = run_bass_kernel_spmd(nc, in_maps, list(range(n_cores)))
    out = np.concatenate(
        [unpack_g(res.results[c]["g"], f, n) for c in range(n_cores)], axis=0
    )
    return out
